# revision 18
# baseline (speedup 1.0000x reference)
"""BBox spatial attention kernel for Trainium2 (8 NeuronCores, data-parallel over B).

Reference math per batch b, box n:
    gauss[n, y, x] = exp(-(dx2[n, x] + dy2[n, y]))
    att[y, x]      = max_n gauss   (all-zero boxes masked out)

exp is monotonic, so att = exp(-min_n (dx2[n,x] + dy2[n,y])). The additive
z[n,y,x] = dy2[n,y] + dx2[n,x] field is rank-2, so each 4-box group of z
tiles is built by ONE K=34 float32r matmul straight into a PSUM bank:
  rows 0..31:  lhsT = DY2'[32, 128] (dy2 + invalid-box penalty), rhs = const
               block-diagonal ones -> routes box m's dy2 column into its own
               128-column block.
  rows 32/33:  lhsT = batch-selector ones pattern, rhs = DXF_b[1, 4096]
               (all boxes' dx2 flattened onto one partition) -> broadcasts
               dx2 across all y partitions.
The min over boxes is a strided reduce_min on the VectorEngine straight out
of PSUM, then one Exp per batch on the ScalarEngine. Invalid (all-zero)
boxes get +1e5 added to dy2 -> exp -> exact 0. feature_map only provides
H/W and is never touched.

Sharding: B=16 -> 2 batches per core, 8 cores, no cross-core comms.
"""

import math

import numpy as np

import concourse.bacc as bacc
import concourse.bass as bass
import concourse.mybir as mybir
import concourse.tile as tile
from concourse.bass_utils import run_bass_kernel_spmd

B, N, H, W = 16, 32, 128, 128
N_CORES = 8
B_LOC = B // N_CORES  # 2 batches per core
EPS = 1e-6
F32 = mybir.dt.float32
F32R = mybir.dt.float32r
AX = mybir.AxisListType
ALU = mybir.AluOpType
ACT = mybir.ActivationFunctionType

_CACHE: dict = {}


def _diag_ones() -> np.ndarray:
    d = np.zeros((N, N * W), dtype=np.float32)
    for r in range(N):
        d[r, r * W : (r + 1) * W] = 1.0
    return d


def _bsel_ones() -> np.ndarray:
    s = np.zeros((B_LOC, B_LOC * W), dtype=np.float32)
    for b in range(B_LOC):
        s[b, b * W : (b + 1) * W] = 1.0
    return s


def build_nc(reps: int = 1):
    nc = bacc.Bacc(
        "TRN2",
        target_bir_lowering=False,
        debug=False,
        enable_asserts=False,
    )
    bb = nc.dram_tensor("bb", [B_LOC, N, 4], F32, kind="ExternalInput")
    att = nc.dram_tensor("att", [B_LOC, H, W], F32, kind="ExternalOutput")
    iota2_dram = nc.inline_tensor(
        np.tile(2.0 * np.arange(W, dtype=np.float32), (N, 1)), name="iota2_const"
    )
    diag_dram = nc.inline_tensor(_diag_ones(), name="diag_const")
    bsel_dram = nc.inline_tensor(_bsel_ones(), name="bsel_const")

    with tile.TileContext(nc) as tc:
        with (
            tc.tile_pool(name="sb", bufs=1) as sb,
            tc.tile_pool(name="psum", bufs=2, space="PSUM") as pp,
        ):
            # tiny warmup activation so the ACT function-table load (~1.3us)
            # happens at t=0 instead of blocking the first real Square
            warm = sb.tile([128, 1], F32, tag="warm")
            nc.vector.memset(warm[:], 0.0)
            nc.scalar.activation(warm[:], warm[:], ACT.Square)

            for _rep in range(reps):
                _body(nc, sb, pp, bb, att, iota2_dram, diag_dram, bsel_dram)

    nc.compile()
    return nc


def _body(nc, sb, pp, bb, att, iota2_dram, diag_dram, bsel_dram):
    # all tiles are 128-partition so every matmul operand sits at base
    # partition 0 (PE tile_position (0, 0))
    bbt = sb.tile([128, B_LOC * 4], F32, tag="bbt")  # [n, (b c)]
    nc.sync.dma_start(
        bbt[0:N, :].rearrange("p (b c) -> p b c", b=B_LOC),
        bb.ap().rearrange("b n c -> n b c"),
    )
    iota2 = sb.tile([128, W], F32, tag="iota2")
    nc.sync.dma_start(iota2[0:N, :], iota2_dram.ap())
    # diag rows 0..31: const block-diagonal ones; rows 32/33: dx2 flat (dyn)
    diag = sb.tile([128, N * W], F32R, tag="diag")
    nc.sync.dma_start(diag[0:N, :], diag_dram.ap().bitcast(F32R))
    # uyp rows 0..31: dy2'; rows 32/33: batch-selector ones pattern (const)
    uyp = sb.tile([128, B_LOC * H], F32R, tag="uyp")
    nc.sync.dma_start(uyp[N : N + B_LOC, :], bsel_dram.ap().bitcast(F32R))

    # --- per-box params, boxes on partitions 0..31, b along free ---
    # pixel coords: clip(floor(v*128), 0, 127); v*128 exact (pow2).
    # floor via round-half magic: a = fl(v + (2^23 - 0.5)) = RNE(v - 0.5)+2^23
    # (exact for v in [0, 2^22) with frac(v) != 0; inputs are uniform [0,1)
    # so v is never an exact integer). b = -max(a, 2^23) clamps negatives,
    # fn = b + 2^23 = -clip(floor(v), 0, inf); upper clip unneeded (v < 128).
    MAGIC = 8388608.0  # 2^23
    a = sb.tile([128, 8], F32, tag="a")
    nc.vector.tensor_scalar(
        a[0:N, :], bbt[0:N, :], float(W), MAGIC - 0.5, ALU.mult, ALU.add
    )
    bm = sb.tile([128, 8], F32, tag="bm")
    nc.vector.tensor_scalar(bm[0:N, :], a[0:N, :], MAGIC, -1.0, ALU.max, ALU.mult)
    # s[:, 2k+b] = hi-lo box extent (from bm directly; the 2^23 offsets cancel)
    bv = bm[0:N, :].rearrange("p (b c) -> p b c", b=B_LOC)
    s = sb.tile([128, 4], F32, tag="s")
    nc.vector.tensor_tensor(
        s[0:N, :].rearrange("p (k b) -> p b k", k=2),
        bv[:, :, 0:2],
        bv[:, :, 2:4],
        ALU.subtract,
    )
    # d = 2*sqrt(2)*(s*0.25 + eps); r2 = 1/d so (2x - c)*r2 = (x-cx)/(sqrt2*sx)
    d = sb.tile([128, 4], F32, tag="d")
    nc.vector.tensor_scalar(
        d[0:N, :],
        s[0:N, :],
        math.sqrt(2.0) / 2.0,
        2.0 * math.sqrt(2.0) * EPS,
        ALU.mult,
        ALU.add,
    )
    r2 = sb.tile([128, 4], F32, tag="r2")
    nc.vector.reciprocal(r2[0:N, :], d[0:N, :])

    # fn = -clip(floor, 0, 127); cn = -(lo+hi) = -c
    fn = sb.tile([128, 8], F32, tag="fn")
    nc.vector.tensor_scalar(fn[0:N, :], bm[0:N, :], MAGIC, None, ALU.add)
    fv = fn[0:N, :].rearrange("p (b c) -> p b c", b=B_LOC)
    cn = sb.tile([128, 4], F32, tag="cn")
    nc.vector.tensor_tensor(
        cn[0:N, :].rearrange("p (k b) -> p b k", k=2),
        fv[:, :, 2:4],
        fv[:, :, 0:2],
        ALU.add,
    )

    # t4 block j = (iota2 + cn_j) * r2_j = (2x - c)/(2*sqrt2*s2); the
    # subtraction happens exactly BEFORE the multiply (avoids catastrophic
    # cancellation for narrow boxes). blocks j = (k, b):
    # [tx b0 | tx b1 | ty b0 | ty b1]. x blocks first -> flatten DMA ASAP.
    t4 = sb.tile([128, 4 * W], F32, tag="t4")
    u4 = sb.tile([128, 4 * W], F32, tag="u4")
    for j in range(4):
        nc.vector.tensor_scalar(
            t4[0:N, j * W : (j + 1) * W],
            iota2[0:N, :],
            cn[0:N, j : j + 1],
            r2[0:N, j : j + 1],
            ALU.add,
            ALU.mult,
        )
        if j == 1:
            nc.scalar.activation(
                u4[0:N, 0 : 2 * W], t4[0:N, 0 : 2 * W], ACT.Square
            )
            for jj in range(2):
                nc.sync.dma_start(
                    diag[N + jj : N + jj + 1, :],
                    u4[0:N, jj * W : (jj + 1) * W].bitcast(F32R),
                )
    nc.scalar.activation(u4[0:N, 2 * W : 4 * W], t4[0:N, 2 * W : 4 * W], ACT.Square)

    # all-zero-box mask -> +1e5 penalty added to dy2 (runs during ACT work)
    s4 = sb.tile([128, 2], F32, tag="s4")
    nc.vector.reduce_sum(
        s4[0:N, :], bbt[0:N, :].rearrange("p (b c) -> p b c", b=B_LOC), axis=AX.X
    )
    pen = sb.tile([128, 2], F32, tag="pen")
    nc.vector.tensor_scalar(
        pen[0:N, :], s4[0:N, :], 0.0, 1.0e5, ALU.is_equal, ALU.mult
    )
    for b in range(B_LOC):
        nc.vector.tensor_scalar(
            uyp[0:N, b * H : (b + 1) * H],
            u4[0:N, (2 + b) * H : (3 + b) * H],
            pen[0:N, b : b + 1],
            None,
            ALU.add,
        )

    # z = dy2' + dx2 in PSUM via one K=34 f32r matmul per 4-box group;
    # strided reduce_min on DVE straight out of PSUM. Chunked (1, 3, 4)
    # groups per batch so the first reduce starts after a single matmul;
    # chunk slots (1+3+4 banks = full PSUM) ping-pong between batches.
    K = N + B_LOC  # 34
    CHUNKS = (1, 3, 4)
    for b in range(B_LOC):
        mns = []
        gbase = 0
        for nch, ngrp in enumerate(CHUNKS):
            pt = pp.tile([H, ngrp * 512], F32, tag=f"pt{nch}", bufs=1)
            for gl in range(ngrp):
                nc.tensor.matmul(
                    pt[:, 512 * gl : 512 * (gl + 1)],
                    uyp[0:K, b * H : (b + 1) * H],
                    diag[0:K, 512 * (gbase + gl) : 512 * (gbase + gl + 1)],
                    start=True,
                    stop=True,
                )
            gbase += ngrp
            mn = sb.tile([H, W], F32, tag=f"mn{nch}")
            nc.vector.tensor_reduce(
                mn[:],
                pt[:].rearrange("p (i x) -> p x i", i=4 * ngrp),
                axis=AX.X,
                op=ALU.min,
            )
            mns.append(mn)
        nma = sb.tile([H, W], F32, tag="nma")
        nc.vector.tensor_tensor(nma[:], mns[0][:], mns[1][:], ALU.min)
        nmb = sb.tile([H, W], F32, tag="nmb")
        nc.vector.tensor_tensor(nmb[:], nma[:], mns[2][:], ALU.min)
        res = sb.tile([H, W], F32, tag="res")
        nc.scalar.activation(res[:], nmb[:], ACT.Exp, scale=-1.0)
        nc.sync.dma_start(att.ap()[b], res[:])


def _get_nc():
    if "nc" not in _CACHE:
        _CACHE["nc"] = build_nc()
    return _CACHE["nc"]


def kernel(feature_map: np.ndarray, bboxes: np.ndarray) -> np.ndarray:
    nc = _get_nc()
    bb = np.ascontiguousarray(bboxes, dtype=np.float32)
    in_maps = [
        {"bb": bb[c * B_LOC : (c + 1) * B_LOC]} for c in range(N_CORES)
    ]
    res = run_bass_kernel_spmd(nc, in_maps, list(range(N_CORES)))
    out = np.concatenate([res.results[c]["att"] for c in range(N_CORES)], axis=0)
    return out[:, None, :, :].astype(np.float32, copy=False)
